# revision 32
# baseline (speedup 1.0000x reference)
"""Trainium2 Bass kernel for nn_CausalAttention_33930241639029.

Sharding: tensor-parallel over heads (8 heads -> 8 cores). Each core computes
RMSNorm + its head's q/k/v projection + RoPE + causal attention + gated memory
retrieval + delta-rule memory update. The output projection is token-sharded:
gated head outputs are exchanged with an AllToAll, after which each core
computes out = merged @ w_out.T for its 512-token chunk.

Numerics: matmuls in fp32r (full-rate fp32) except P@V and the memory
retrieval for q (bf16 - both are benign), and the delta-rule path (fp32).
Softmax runs without max-subtraction (logits are provably in [-4, 4] for
RMS-normed inputs with 0.02-scaled weights); the denominator comes for free
from a ones-column appended to V.
"""

import numpy as np
import ml_dtypes


class _StageDone(Exception):
    def __init__(self, nc, tc=None):
        self.nc = nc
        self.tc = tc


B, N, D, H, DH = 2, 2048, 1024, 8, 128
NT = B * N  # 4096 flattened tokens (b-major)
SCALE = DH ** -0.5
N_CORES = 8
ROPE_THETA = 10000.0

_cache = {}


def _host_constants():
    # RoPE tables, transposed to [dh, n] layout (lucidrains interleaved freqs).
    inv_freq = 1.0 / (ROPE_THETA ** (np.arange(0, DH, 2, dtype=np.float64) / DH))
    freqs = np.arange(N, dtype=np.float64)[:, None] * inv_freq[None, :]  # [n, dh/2]
    freqs = np.repeat(freqs, 2, axis=-1)  # [n, dh]
    cosT = np.ascontiguousarray(np.cos(freqs).T.astype(np.float32))  # [dh, n]
    sinT = np.ascontiguousarray(np.sin(freqs).T.astype(np.float32))
    # rotate_half as a matmul stationary: out[2i] = -q[2i+1], out[2i+1] = q[2i]
    # lhsT[d, p] with out[p, t] = sum_d lhsT[d, p] * q[d, t]
    pc = np.zeros((DH, DH), dtype=np.float32)
    idx = np.arange(0, DH, 2)
    pc[idx + 1, idx] = -1.0
    pc[idx, idx + 1] = 1.0
    # causal keep-mask in [j, i] layout: keep i >= j
    jj = np.arange(DH)[:, None]
    ii = np.arange(DH)[None, :]
    masku = (ii >= jj).astype(ml_dtypes.bfloat16)
    ident = np.eye(DH, dtype=np.float32)
    return cosT, sinT, pc, masku, ident


def _build_nc(stage=99):
    import concourse.bass as bass
    import concourse.tile as tile
    from concourse import bacc, mybir
    from concourse.bass import ts

    f32 = mybir.dt.float32
    bf16 = mybir.dt.bfloat16
    r32 = mybir.dt.float32r
    AX = mybir.AxisListType
    AF = mybir.ActivationFunctionType
    OP = mybir.AluOpType

    nc = bacc.Bacc("TRN2", target_bir_lowering=False, debug=False,
                   num_devices=N_CORES)

    # ---- DRAM I/O ----
    x_d = nc.dram_tensor("x", [NT, D], f32, kind="ExternalInput")
    wT_d = nc.dram_tensor("wT", [D, 3 * DH], f32, kind="ExternalInput")
    gamma_d = nc.dram_tensor("gamma", [D, 1], f32, kind="ExternalInput")
    woT_d = nc.dram_tensor("w_outT", [D, D], r32, kind="ExternalInput")
    cosT_d = nc.dram_tensor("cosT", [DH, N], f32, kind="ExternalInput")
    sinT_d = nc.dram_tensor("sinT", [DH, N], f32, kind="ExternalInput")
    pc_d = nc.dram_tensor("pc", [DH, DH], r32, kind="ExternalInput")
    masku_d = nc.dram_tensor("masku", [DH, DH], bf16, kind="ExternalInput")
    id_d = nc.dram_tensor("ident", [DH, DH], f32, kind="ExternalInput")
    mem_d = nc.dram_tensor("mem_aug", [B, DH, 132], f32, kind="ExternalInput")
    hg_d = nc.dram_tensor("hg", [1, 1], f32, kind="ExternalInput")

    out_d = nc.dram_tensor("out", [NT // N_CORES, D], f32, kind="ExternalOutput")
    nkv_d = nc.dram_tensor("new_kv", [B, DH, DH], f32, kind="ExternalOutput")
    nnm_d = nc.dram_tensor("new_norm", [B, DH, 1], f32, kind="ExternalOutput")

    def _emit(tc):
        # pools with hand-managed lifetimes (open/close in program order)
        _cms = {}

        def popen(name, **kw):
            c = tc.tile_pool(name=name, **kw)
            _cms[name] = c
            return c.__enter__()

        def pclose(*names):
            for n in names:
                _cms.pop(n).__exit__(None, None, None)

        dram = popen("dram", bufs=1, space="DRAM")
        xn_dram = [dram.tile([512, D], bf16, name=f"xnd{t}") for t in range(8)]
        a2a_in = [dram.tile([D, N // N_CORES], r32, name=f"a2ai{b}")
                  for b in range(B)]
        a2a_out = [dram.tile([D, N // N_CORES], r32, name=f"a2ao{b}")
                   for b in range(B)]

        consts = popen("consts", bufs=1)
        rope_pool = popen("rope", bufs=1, side="right")
        wtg_pool = popen("wtg", bufs=1, side="right")

        cosT = rope_pool.tile([DH, N], f32)
        sinT = rope_pool.tile([DH, N], f32)
        nc.sync.dma_start(out=cosT[:], in_=cosT_d[:])
        nc.sync.dma_start(out=sinT[:], in_=sinT_d[:])
        pc_sb = consts.tile([DH, DH], r32)
        nc.sync.dma_start(out=pc_sb[:], in_=pc_d[:])
        masku = consts.tile([DH, DH], bf16)
        nc.sync.dma_start(out=masku[:], in_=masku_d[:])
        id_sb = consts.tile([DH, DH], f32)
        nc.sync.dma_start(out=id_sb[:], in_=id_d[:])
        mem_sb = consts.tile([DH, B, 132], f32)
        for b in range(B):
            nc.sync.dma_start(out=mem_sb[:, b, :], in_=mem_d[b])
        mem_bf = consts.tile([DH, B, 132], bf16)
        nc.vector.tensor_copy(mem_bf[:], mem_sb[:])

        # gate g = sigmoid(head_gate), broadcast to [128, 1] via PE
        hg_sb = consts.tile([1, 1], f32)
        nc.sync.dma_start(out=hg_sb[:], in_=hg_d[:])
        g_sb = consts.tile([1, 1], f32)
        nc.scalar.activation(g_sb[:], hg_sb[:], AF.Sigmoid)
        ones1 = consts.tile([1, DH], f32)
        nc.vector.memset(ones1[:], 1.0)
        with tc.tile_pool(name="ps_g", bufs=1, space="PSUM") as ps_g:
            gp = ps_g.tile([DH, 1], f32)
            nc.tensor.matmul(gp[:], ones1[:], g_sb[:], start=True, stop=True)
            g128 = consts.tile([DH, 1], f32)
            nc.vector.tensor_copy(g128[:], gp[:])
        om128 = consts.tile([DH, 1], f32)
        nc.vector.tensor_scalar(om128[:], g128[:], -1.0, 1.0, OP.mult, OP.add)

        # gamma-folded, bf16-cast weights: wTg[:, ci, :] = wT[ci*128:, :] * gamma
        wTg = wtg_pool.tile([DH, 8, 3 * DH], bf16)
        with tc.tile_pool(name="wload", bufs=2) as wload:
            for ci in range(8):
                wt = wload.tile([DH, 3 * DH], f32, tag="wt")
                nc.sync.dma_start(out=wt[:], in_=wT_d[ts(ci, DH), :])
                gt = wload.tile([DH, 1], f32, tag="gt")
                nc.sync.dma_start(out=gt[:], in_=gamma_d[ts(ci, DH), :])
                nc.scalar.activation(wTg[:, ci, :], wt[:], AF.Copy, scale=gt[:])

        # ---- Phases 1-3 fused, pipelined per 512-token chunk ----
        # per chunk: load x rows -> RMSNorm -> xn(bf16) to DRAM -> DMA-transpose
        # -> qkv matmuls -> epilogues (rope, features) -> v/kf transposes
        xnt_pool = popen("xnt", bufs=3, side="right")
        big = popen("big", bufs=1)
        qsT = [big.tile([DH, 512], r32, tag=f"qsT{t}", name=f"qsT{t}")
               for t in range(8)]
        kT = [big.tile([DH, 512], r32, tag=f"kT{t}", name=f"kT{t}")
              for t in range(8)]
        qfT = big.tile([DH, NT], bf16)
        kfT = big.tile([DH, NT], f32)
        nacc = big.tile([DH, 8], f32)
        nat = popen("nat", bufs=1)
        v_nat = nat.tile([DH, NT // DH, DH], f32)
        v_aug = nat.tile([DH, NT // DH, 132], bf16)
        kf_nat = nat.tile([DH, NT // DH, DH], f32)
        nc.vector.memset(v_aug[:], 1.0)

        with tc.tile_pool(name="p1", bufs=4) as p1, \
             tc.tile_pool(name="p1x", bufs=5) as p1x, \
             tc.tile_pool(name="p1n", bufs=4) as p1n, \
             tc.tile_pool(name="p1s", bufs=2) as p1s, \
             tc.tile_pool(name="ps_qkv", bufs=4, space="PSUM") as ps_qkv, \
             tc.tile_pool(name="ps_rot", bufs=2, space="PSUM") as ps_rot, \
             tc.tile_pool(name="ps_tr", bufs=2, space="PSUM") as ps_tr, \
             tc.tile_pool(name="p3t", bufs=2) as p3t, \
             tc.tile_pool(name="vtp", bufs=2) as vtp:
            for t in range(8):
                t0 = t * 512
                n0 = (t % 4) * 512  # position offset within batch
                # -- RMSNorm for the 4 x-tiles of this chunk --
                ssc = p1.tile([DH, 4], f32, tag="ssc")
                xts = []
                for ii in range(4):
                    i = t * 4 + ii
                    xt = p1x.tile([DH, D], f32, tag="xt")
                    xts.append(xt)
                    nc.sync.dma_start(out=xt[:], in_=x_d[ts(i, DH), :])
                    sq = p1s.tile([DH, D], f32, tag="sq")
                    nc.vector.scalar_tensor_tensor(
                        out=sq[:], in0=xt[:], scalar=1.0, in1=xt[:],
                        op0=OP.mult, op1=OP.mult, accum_out=ssc[:, ii:ii + 1])
                rt4 = p1.tile([DH, 4], f32, tag="rt4")
                nc.scalar.activation(rt4[:], ssc[:], AF.Sqrt, scale=1.0 / D)
                rc4 = p1.tile([DH, 4], f32, tag="rc4")
                nc.vector.reciprocal(rc4[:], rt4[:])
                for ii in range(4):
                    i = t * 4 + ii
                    xnb = p1n.tile([DH, D], bf16, tag="xnb")
                    nc.scalar.activation(xnb[:], xts[ii][:], AF.Copy,
                                         scale=rc4[:, ii:ii + 1])
                    nc.sync.dma_start(out=xn_dram[t][ts(ii, DH), :], in_=xnb[:])
                # -- DMA-transpose this chunk: [512, 128] -> [128, 512] x8 --
                xnT = xnt_pool.tile([DH, 8, 512], bf16, tag="xnT")
                for ci in range(8):
                    nc.sync.dma_start_transpose(
                        out=xnT[:, ci, :],
                        in_=xn_dram[t][:, ts(ci, DH)])
                # -- qkv matmuls --
                pq = {}
                for blk in range(3):  # 0=q 1=k 2=v
                    ps = ps_qkv.tile([DH, 512], f32, tag="qkv")
                    pq[blk] = ps
                    for ci in range(8):
                        nc.tensor.matmul(
                            ps[:], wTg[:, ci, ts(blk, DH)],
                            xnT[:, ci, :],
                            start=(ci == 0), stop=(ci == 7))
                # q: scaled copy; k: raw copy; v: copy for transposing
                nc.scalar.activation(qsT[t][:], pq[0][:], AF.Copy, scale=SCALE)
                nc.scalar.activation(kT[t][:], pq[1][:], AF.Copy)
                vT = vtp.tile([DH, 512], f32, tag="vT")
                nc.scalar.activation(vT[:], pq[2][:], AF.Copy)
                # feature maps: f(x) = relu(x) + exp(min(x, 0))
                for blk, dst in ((0, qfT), (1, kfT)):
                    src = pq[blk]
                    mn = p3t.tile([DH, 512], f32, tag="mn")
                    nc.vector.tensor_scalar(mn[:], src[:], 0.0, None, OP.min)
                    ex = p3t.tile([DH, 512], f32, tag="ex")
                    nc.scalar.activation(ex[:], mn[:], AF.Exp)
                    rl = p3t.tile([DH, 512], f32, tag="rl")
                    nc.scalar.activation(rl[:], src[:], AF.Relu)
                    acc = nacc[:, t:t + 1] if blk == 1 else None
                    nc.vector.scalar_tensor_tensor(
                        out=dst[:, t0:t0 + 512], in0=ex[:], scalar=1.0,
                        in1=rl[:], op0=OP.mult, op1=OP.add, accum_out=acc)
                # rope (in-place): rot = Pc @ src ; src = src*cos + rot*sin
                for srcT in (qsT[t], kT[t]):
                    rp = ps_rot.tile([DH, 512], f32, tag="rot")
                    nc.tensor.matmul(rp[:], pc_sb[:], srcT[:], start=True,
                                     stop=True)
                    t1 = p3t.tile([DH, 512], f32, tag="t1")
                    nc.gpsimd.tensor_mul(t1[:], srcT[:].bitcast(f32),
                                         cosT[:, n0:n0 + 512])
                    m2 = p3t.tile([DH, 512], f32, tag="m2")
                    nc.vector.tensor_mul(m2[:], rp[:], sinT[:, n0:n0 + 512])
                    nc.gpsimd.tensor_add(srcT[:], t1[:], m2[:])
                # v and kf transposes for this chunk (4 blocks each)
                for bi in range(4):
                    tb = t * 4 + bi
                    tp = ps_tr.tile([DH, DH], f32, tag="tr")
                    nc.tensor.transpose(tp[:], vT[:, ts(bi, DH)], id_sb[:])
                    nc.vector.tensor_copy(v_nat[:, tb, :], tp[:])
                    nc.scalar.activation(v_aug[:, tb, 0:DH], tp[:], AF.Copy)
                    tp2 = ps_tr.tile([DH, DH], f32, tag="tr")
                    nc.tensor.transpose(tp2[:], kfT[:, ts(tb, DH)], id_sb[:])
                    nc.vector.tensor_copy(kf_nat[:, tb, :], tp2[:])

        pclose("xnt")
        pclose("wtg", "rope")
        if stage < 4:
            pclose("nat", "big", "consts", "dram")
            return

        # ---- Phase 4: causal attention + gated retrieval, per batch ----
        ogt_pool = popen("ogt", bufs=1, side="right")
        ogT = ogt_pool.tile([DH, NT], r32)
        NB = N // DH  # 16 key/query blocks per batch

        with tc.tile_pool(name="expp", bufs=1) as expp, \
             tc.tile_pool(name="ps_sim", bufs=2, space="PSUM") as ps_sim, \
             tc.tile_pool(name="ps_pv", bufs=2, space="PSUM") as ps_pv, \
             tc.tile_pool(name="ps_rt", bufs=1, space="PSUM") as ps_rt, \
             tc.tile_pool(name="ps_tr2", bufs=1, space="PSUM") as ps_tr2, \
             tc.tile_pool(name="p4t", bufs=3) as p4t:
            for b in range(B):
                Bo = b * N
                expt = {}
                for jb in range(NB):
                    al = (jb // 4) * 512
                    d0 = jb * DH
                    W = N - d0
                    et = expp.tile([DH, W], bf16, tag=f"exp{jb}", name=f"exp{jb}")
                    expt[jb] = (et, d0)
                    kblk = kT[(Bo + jb * DH) // 512][:, ts(jb % 4, DH)]
                    for c0 in range(al, N, 1024):
                        cw = min(1024, N - c0)
                        ps = ps_sim.tile([DH, cw], f32, tag="sim")
                        for s0 in range(c0, c0 + cw, 512):
                            qch = qsT[(Bo + s0) // 512]
                            nc.tensor.matmul(
                                ps[:, s0 - c0:s0 - c0 + 512],
                                kblk, qch[:],
                                start=True, stop=True)
                        e0 = max(c0, d0)
                        nc.scalar.activation(et[:, e0 - d0:c0 + cw - d0],
                                             ps[:, e0 - c0:cw], AF.Exp)
                    nc.gpsimd.tensor_mul(et[:, 0:DH], et[:, 0:DH], masku[:])
                for ib in range(NB):
                    pv = ps_pv.tile([DH, 132], f32, tag="pv")
                    for jb in range(ib + 1):
                        et, d0 = expt[jb]
                        nc.tensor.matmul(
                            pv[:, 0:129], et[:, ib * DH - d0:ib * DH - d0 + DH],
                            v_aug[:, b * NB + jb, 0:129],
                            start=(jb == 0), stop=(jb == ib))
                    rt = ps_rt.tile([DH, 132], f32, tag="rt")
                    nc.tensor.matmul(rt[:, 0:129],
                                     qfT[:, Bo + ib * DH:Bo + (ib + 1) * DH],
                                     mem_bf[:, b, 0:129], start=True, stop=True)
                    d1 = p4t.tile([DH, 1], f32, tag="d1")
                    nc.vector.reciprocal(d1[:], pv[:, 128:129])
                    s1 = p4t.tile([DH, 1], f32, tag="s1")
                    nc.vector.tensor_mul(s1[:], d1[:], g128[:])
                    dm = p4t.tile([DH, 1], f32, tag="dm")
                    nc.vector.tensor_scalar(dm[:], rt[:, 128:129], 1e-10, None,
                                            OP.max)
                    d2 = p4t.tile([DH, 1], f32, tag="d2")
                    nc.vector.reciprocal(d2[:], dm[:])
                    s2 = p4t.tile([DH, 1], f32, tag="s2")
                    nc.vector.tensor_mul(s2[:], d2[:], om128[:])
                    t1g = p4t.tile([DH, DH], f32, tag="t1g")
                    nc.vector.tensor_scalar(t1g[:], pv[:, 0:DH], s1[:], None,
                                            OP.mult)
                    og = p4t.tile([DH, DH], f32, tag="og")
                    nc.vector.scalar_tensor_tensor(
                        out=og[:], in0=rt[:, 0:DH], scalar=s2[:], in1=t1g[:],
                        op0=OP.mult, op1=OP.add)
                    tp = ps_tr2.tile([DH, DH], f32, tag="ogtr")
                    nc.tensor.transpose(tp[:], og[:], id_sb[:])
                    nc.vector.tensor_copy(
                        ogT[:, Bo + ib * DH:Bo + (ib + 1) * DH], tp[:])
                if stage >= 6:
                    HCH = N // N_CORES
                    for j in range(N_CORES):
                        nc.sync.dma_start(
                            out=a2a_in[b][ts(j, DH), :],
                            in_=ogT[:, Bo + j * HCH:Bo + (j + 1) * HCH])
                    if stage >= 7:
                        nc.gpsimd.collective_compute(
                            "AllToAll", OP.bypass,
                            replica_groups=[list(range(N_CORES))],
                            ins=[a2a_in[b].opt()], outs=[a2a_out[b].opt()])
                    else:
                        with tc.tile_pool(name=f"bounce{b}", bufs=2) as bounce:
                            for j in range(N_CORES):
                                bt = bounce.tile([DH, HCH], r32, tag="bt")
                                nc.sync.dma_start(out=bt[:],
                                                  in_=a2a_in[b][ts(j, DH), :])
                                nc.sync.dma_start(out=a2a_out[b][ts(j, DH), :],
                                                  in_=bt[:])

        if stage < 5:
            pclose("nat", "big", "ogt", "consts", "dram")
            return
        # ---- Phase 5: delta-rule memory update, per batch ----
        with tc.tile_pool(name="ps_dr", bufs=2, space="PSUM") as ps_dr, \
             tc.tile_pool(name="ps_nk", bufs=1, space="PSUM") as ps_nk, \
             tc.tile_pool(name="p5t", bufs=3) as p5t:
            for b in range(B):
                Bo = b * N
                nk = ps_nk.tile([DH, DH], f32, tag="nk")
                for tb in range(NB):
                    g = b * NB + tb
                    dr = ps_dr.tile([DH, 132], f32, tag="dr")
                    nc.tensor.matmul(dr[:, 0:129],
                                     kfT[:, Bo + tb * DH:Bo + (tb + 1) * DH],
                                     mem_sb[:, b, 0:129], start=True, stop=True)
                    dm = p5t.tile([DH, 1], f32, tag="dm5")
                    nc.vector.tensor_scalar(dm[:], dr[:, 128:129], 1e-10, None,
                                            OP.max)
                    rd = p5t.tile([DH, 1], f32, tag="rd5")
                    nc.vector.reciprocal(rd[:], dm[:])
                    rdn = p5t.tile([DH, 1], f32, tag="rdn5")
                    nc.vector.tensor_scalar(rdn[:], rd[:], -1.0, None, OP.mult)
                    vn = p5t.tile([DH, DH], f32, tag="vn")
                    nc.vector.scalar_tensor_tensor(
                        out=vn[:], in0=dr[:, 0:DH], scalar=rdn[:],
                        in1=v_nat[:, g, :], op0=OP.mult, op1=OP.add)
                    nc.tensor.matmul(nk[:], kf_nat[:, g, :], vn[:],
                                     start=(tb == 0), stop=(tb == NB - 1),
                                     skip_group_check=True)
                nkv = p5t.tile([DH, DH], f32, tag="nkv")
                nc.vector.tensor_add(nkv[:], nk[:], mem_sb[:, b, 0:DH])
                nc.sync.dma_start(out=nkv_d[b], in_=nkv[:])
                nsum = p5t.tile([DH, 1], f32, tag="nsum")
                nc.vector.tensor_reduce(nsum[:], nacc[:, b * 4:(b + 1) * 4],
                                        AX.X, OP.add)
                nn2 = p5t.tile([DH, 1], f32, tag="nn2")
                nc.vector.tensor_add(nn2[:], nsum[:], mem_sb[:, b, 128:129])
                nc.sync.dma_start(out=nnm_d[b], in_=nn2[:])

        pclose("nat", "big")

        if stage < 6:
            pclose("ogt", "consts", "dram")
            return
        # ---- Phase 6: token-sharded output projection ----
        HCH = N // N_CORES  # 256 tokens per core per batch
        fin = popen("fin", bufs=1)
        wo = fin.tile([DH, 8, D], r32)
        for ci in range(8):
            nc.sync.dma_start(out=wo[:, ci, :], in_=woT_d[ts(ci, DH), :])
        mg = fin.tile([DH, B, 8, HCH], r32)
        for b in range(B):
            for c in range(8):
                nc.sync.dma_start(out=mg[:, b, c, :],
                                  in_=a2a_out[b][ts(c, DH), :])
        with tc.tile_pool(name="ps_fin", bufs=2, space="PSUM") as ps_fin, \
             tc.tile_pool(name="p6t", bufs=2) as p6t:
            for b in range(B):
                for tb in range(HCH // DH):
                    fp = ps_fin.tile([DH, D], f32, tag="fin")
                    for ci in range(8):
                        for o0 in range(0, D, 512):
                            nc.tensor.matmul(
                                fp[:, o0:o0 + 512],
                                mg[:, b, ci, ts(tb, DH)],
                                wo[:, ci, o0:o0 + 512],
                                start=(ci == 0), stop=(ci == 7),
                                skip_group_check=True)
                    ot = p6t.tile([DH, D], f32, tag="ot")
                    nc.vector.tensor_copy(ot[:], fp[:])
                    nc.sync.dma_start(out=out_d[ts(b * 2 + tb, DH), :], in_=ot[:])

        pclose("fin", "ogt", "consts", "dram")

    with tile.TileContext(nc) as tc:
        _emit(tc)
    nc.compile()
    return nc


def _get_nc():
    if "nc" not in _cache:
        import os
        stage = int(os.environ.get("BASS_KERNEL_STAGE", "99"))
        try:
            _cache["nc"] = _build_nc(stage)
        except _StageDone as e:
            raise RuntimeError("unreachable")
    return _cache["nc"]


def kernel(x, gamma, w_qkv, w_out, head_gates, mem_kv, mem_norm):
    from concourse.bass_utils import run_bass_kernel_spmd

    x = np.asarray(x, dtype=np.float32)
    gamma = np.asarray(gamma, dtype=np.float32)
    w_qkv = np.asarray(w_qkv, dtype=np.float32)
    w_out = np.asarray(w_out, dtype=np.float32)
    head_gates = np.asarray(head_gates, dtype=np.float32)
    mem_kv = np.asarray(mem_kv, dtype=np.float32)
    mem_norm = np.asarray(mem_norm, dtype=np.float32)

    nc = _get_nc()
    cosT, sinT, pc, masku, ident = _host_constants()

    x_flat = np.ascontiguousarray(x.reshape(NT, D))
    gamma2 = np.ascontiguousarray(gamma.reshape(D, 1))
    woT = np.ascontiguousarray(w_out.T)

    in_maps = []
    for h in range(N_CORES):
        wq = w_qkv[h * DH:(h + 1) * DH]
        wk = w_qkv[D + h * DH:D + (h + 1) * DH]
        wv = w_qkv[2 * D + h * DH:2 * D + (h + 1) * DH]
        wT = np.ascontiguousarray(np.concatenate([wq, wk, wv], 0).T)  # [D, 384]
        mem_aug = np.zeros((B, DH, 132), dtype=np.float32)
        mem_aug[:, :, 0:DH] = mem_kv[:, h]
        mem_aug[:, :, DH] = mem_norm[:, h]
        in_maps.append({
            "x": x_flat, "wT": wT, "gamma": gamma2, "w_outT": woT,
            "cosT": cosT, "sinT": sinT, "pc": pc, "masku": masku,
            "ident": ident, "mem_aug": mem_aug,
            "hg": head_gates[h].reshape(1, 1),
        })

    res = run_bass_kernel_spmd(nc, in_maps, core_ids=list(range(N_CORES)),
                               **_cache.get("run_kwargs", {}))
    _cache["last_result"] = res

    out = np.empty((NT, D), dtype=np.float32)
    new_kv = np.empty((B, H, DH, DH), dtype=np.float32)
    new_norm = np.empty((B, H, DH), dtype=np.float32)
    HCH = N // N_CORES
    for h in range(N_CORES):
        r = res.results[h]
        out[h * HCH:(h + 1) * HCH] = r["out"][0:HCH]
        out[N + h * HCH:N + (h + 1) * HCH] = r["out"][HCH:2 * HCH]
        new_kv[:, h] = r["new_kv"]
        new_norm[:, h] = r["new_norm"].reshape(B, DH)
    return out.reshape(B, N, D), new_kv, new_norm


# revision 34
# speedup vs baseline: 1.0334x; 1.0334x over previous
"""Trainium2 Bass kernel for nn_CausalAttention_33930241639029.

Sharding: tensor-parallel over heads (8 heads -> 8 cores). Each core computes
RMSNorm + its head's q/k/v projection + RoPE + causal attention + gated memory
retrieval + delta-rule memory update. The output projection is token-sharded:
gated head outputs are exchanged with an AllToAll, after which each core
computes out = merged @ w_out.T for its 512-token chunk.

Numerics: matmuls in fp32r (full-rate fp32) except P@V and the memory
retrieval for q (bf16 - both are benign), and the delta-rule path (fp32).
Softmax runs without max-subtraction (logits are provably in [-4, 4] for
RMS-normed inputs with 0.02-scaled weights); the denominator comes for free
from a ones-column appended to V.
"""

import numpy as np
import ml_dtypes


class _StageDone(Exception):
    def __init__(self, nc, tc=None):
        self.nc = nc
        self.tc = tc


B, N, D, H, DH = 2, 2048, 1024, 8, 128
NT = B * N  # 4096 flattened tokens (b-major)
SCALE = DH ** -0.5
N_CORES = 8
ROPE_THETA = 10000.0

_cache = {}


def _host_constants():
    # RoPE tables, transposed to [dh, n] layout (lucidrains interleaved freqs).
    inv_freq = 1.0 / (ROPE_THETA ** (np.arange(0, DH, 2, dtype=np.float64) / DH))
    freqs = np.arange(N, dtype=np.float64)[:, None] * inv_freq[None, :]  # [n, dh/2]
    freqs = np.repeat(freqs, 2, axis=-1)  # [n, dh]
    cosT = np.ascontiguousarray(np.cos(freqs).T.astype(np.float32))  # [dh, n]
    sinT = np.ascontiguousarray(np.sin(freqs).T.astype(np.float32))
    # rotate_half as a matmul stationary: out[2i] = -q[2i+1], out[2i+1] = q[2i]
    # lhsT[d, p] with out[p, t] = sum_d lhsT[d, p] * q[d, t]
    pc = np.zeros((DH, DH), dtype=np.float32)
    idx = np.arange(0, DH, 2)
    pc[idx + 1, idx] = -1.0
    pc[idx, idx + 1] = 1.0
    # causal keep-mask in [j, i] layout: keep i >= j
    jj = np.arange(DH)[:, None]
    ii = np.arange(DH)[None, :]
    masku = (ii >= jj).astype(ml_dtypes.bfloat16)
    ident = np.eye(DH, dtype=np.float32)
    return cosT, sinT, pc, masku, ident


def _build_nc(stage=99):
    import concourse.bass as bass
    import concourse.tile as tile
    from concourse import bacc, mybir
    from concourse.bass import ts

    f32 = mybir.dt.float32
    bf16 = mybir.dt.bfloat16
    r32 = mybir.dt.float32r
    AX = mybir.AxisListType
    AF = mybir.ActivationFunctionType
    OP = mybir.AluOpType

    nc = bacc.Bacc("TRN2", target_bir_lowering=False, debug=False,
                   num_devices=N_CORES)

    # ---- DRAM I/O ----
    x_d = nc.dram_tensor("x", [NT, D], f32, kind="ExternalInput")
    wT_d = nc.dram_tensor("wT", [D, 3 * DH], f32, kind="ExternalInput")
    gamma_d = nc.dram_tensor("gamma", [D, 1], f32, kind="ExternalInput")
    woT_d = nc.dram_tensor("w_outT", [D, D], f32, kind="ExternalInput")
    cosT_d = nc.dram_tensor("cosT", [DH, N], f32, kind="ExternalInput")
    sinT_d = nc.dram_tensor("sinT", [DH, N], f32, kind="ExternalInput")
    pc_d = nc.dram_tensor("pc", [DH, DH], r32, kind="ExternalInput")
    masku_d = nc.dram_tensor("masku", [DH, DH], bf16, kind="ExternalInput")
    id_d = nc.dram_tensor("ident", [DH, DH], f32, kind="ExternalInput")
    mem_d = nc.dram_tensor("mem_aug", [B, DH, 132], f32, kind="ExternalInput")
    hg_d = nc.dram_tensor("hg", [1, 1], f32, kind="ExternalInput")

    out_d = nc.dram_tensor("out", [NT // N_CORES, D], f32, kind="ExternalOutput")
    nkv_d = nc.dram_tensor("new_kv", [B, DH, DH], f32, kind="ExternalOutput")
    nnm_d = nc.dram_tensor("new_norm", [B, DH, 1], f32, kind="ExternalOutput")

    def _emit(tc):
        # pools with hand-managed lifetimes (open/close in program order)
        _cms = {}

        def popen(name, **kw):
            c = tc.tile_pool(name=name, **kw)
            _cms[name] = c
            return c.__enter__()

        def pclose(*names):
            for n in names:
                _cms.pop(n).__exit__(None, None, None)

        dram = popen("dram", bufs=1, space="DRAM")
        xn_dram = [dram.tile([512, D], bf16, name=f"xnd{t}") for t in range(8)]
        a2a_in = [dram.tile([D, N // N_CORES], bf16, name=f"a2ai{b}")
                  for b in range(B)]
        a2a_out = [dram.tile([D, N // N_CORES], bf16, name=f"a2ao{b}")
                   for b in range(B)]

        consts = popen("consts", bufs=1)
        rope_pool = popen("rope", bufs=1, side="right")
        wtg_pool = popen("wtg", bufs=1, side="right")

        cosT = rope_pool.tile([DH, N], f32)
        sinT = rope_pool.tile([DH, N], f32)
        nc.sync.dma_start(out=cosT[:], in_=cosT_d[:])
        nc.sync.dma_start(out=sinT[:], in_=sinT_d[:])
        pc_sb = consts.tile([DH, DH], r32)
        nc.sync.dma_start(out=pc_sb[:], in_=pc_d[:])
        masku = consts.tile([DH, DH], bf16)
        nc.sync.dma_start(out=masku[:], in_=masku_d[:])
        id_sb = consts.tile([DH, DH], f32)
        nc.sync.dma_start(out=id_sb[:], in_=id_d[:])
        mem_sb = consts.tile([DH, B, 132], f32)
        for b in range(B):
            nc.sync.dma_start(out=mem_sb[:, b, :], in_=mem_d[b])
        mem_bf = consts.tile([DH, B, 132], bf16)
        nc.vector.tensor_copy(mem_bf[:], mem_sb[:])

        # gate g = sigmoid(head_gate), broadcast to [128, 1] via PE
        hg_sb = consts.tile([1, 1], f32)
        nc.sync.dma_start(out=hg_sb[:], in_=hg_d[:])
        g_sb = consts.tile([1, 1], f32)
        nc.scalar.activation(g_sb[:], hg_sb[:], AF.Sigmoid)
        ones1 = consts.tile([1, DH], f32)
        nc.vector.memset(ones1[:], 1.0)
        with tc.tile_pool(name="ps_g", bufs=1, space="PSUM") as ps_g:
            gp = ps_g.tile([DH, 1], f32)
            nc.tensor.matmul(gp[:], ones1[:], g_sb[:], start=True, stop=True)
            g128 = consts.tile([DH, 1], f32)
            nc.vector.tensor_copy(g128[:], gp[:])
        om128 = consts.tile([DH, 1], f32)
        nc.vector.tensor_scalar(om128[:], g128[:], -1.0, 1.0, OP.mult, OP.add)

        # gamma-folded, bf16-cast weights: wTg[:, ci, :] = wT[ci*128:, :] * gamma
        wTg = wtg_pool.tile([DH, 8, 3 * DH], bf16)
        with tc.tile_pool(name="wload", bufs=2) as wload:
            for ci in range(8):
                wt = wload.tile([DH, 3 * DH], f32, tag="wt")
                nc.sync.dma_start(out=wt[:], in_=wT_d[ts(ci, DH), :])
                gt = wload.tile([DH, 1], f32, tag="gt")
                nc.sync.dma_start(out=gt[:], in_=gamma_d[ts(ci, DH), :])
                nc.scalar.activation(wTg[:, ci, :], wt[:], AF.Copy, scale=gt[:])

        # ---- Phases 1-3 fused, pipelined per 512-token chunk ----
        # per chunk: load x rows -> RMSNorm -> xn(bf16) to DRAM -> DMA-transpose
        # -> qkv matmuls -> epilogues (rope, features) -> v/kf transposes
        xnt_pool = popen("xnt", bufs=3, side="right")
        big = popen("big", bufs=1)
        qsT = [big.tile([DH, 512], r32, tag=f"qsT{t}", name=f"qsT{t}")
               for t in range(8)]
        kT = [big.tile([DH, 512], r32, tag=f"kT{t}", name=f"kT{t}")
              for t in range(8)]
        qfT = big.tile([DH, NT], bf16)
        kfT = big.tile([DH, NT], f32)
        nacc = big.tile([DH, 8], f32)
        nat = popen("nat", bufs=1)
        v_nat = nat.tile([DH, NT // DH, DH], f32)
        v_aug = nat.tile([DH, NT // DH, 132], bf16)
        kf_nat = nat.tile([DH, NT // DH, DH], f32)
        nc.vector.memset(v_aug[:], 1.0)

        with tc.tile_pool(name="p1", bufs=4) as p1, \
             tc.tile_pool(name="p1x", bufs=5) as p1x, \
             tc.tile_pool(name="p1n", bufs=4) as p1n, \
             tc.tile_pool(name="p1s", bufs=2) as p1s, \
             tc.tile_pool(name="ps_qkv", bufs=4, space="PSUM") as ps_qkv, \
             tc.tile_pool(name="ps_rot", bufs=2, space="PSUM") as ps_rot, \
             tc.tile_pool(name="ps_tr", bufs=2, space="PSUM") as ps_tr, \
             tc.tile_pool(name="p3t", bufs=2) as p3t, \
             tc.tile_pool(name="vtp", bufs=2) as vtp:
            for t in range(8):
                t0 = t * 512
                n0 = (t % 4) * 512  # position offset within batch
                # -- RMSNorm for the 4 x-tiles of this chunk --
                ssc = p1.tile([DH, 4], f32, tag="ssc")
                xts = []
                for ii in range(4):
                    i = t * 4 + ii
                    xt = p1x.tile([DH, D], f32, tag="xt")
                    xts.append(xt)
                    nc.sync.dma_start(out=xt[:], in_=x_d[ts(i, DH), :])
                    sq = p1s.tile([DH, D], f32, tag="sq")
                    nc.vector.scalar_tensor_tensor(
                        out=sq[:], in0=xt[:], scalar=1.0, in1=xt[:],
                        op0=OP.mult, op1=OP.mult, accum_out=ssc[:, ii:ii + 1])
                rt4 = p1.tile([DH, 4], f32, tag="rt4")
                nc.scalar.activation(rt4[:], ssc[:], AF.Sqrt, scale=1.0 / D)
                rc4 = p1.tile([DH, 4], f32, tag="rc4")
                nc.vector.reciprocal(rc4[:], rt4[:])
                for ii in range(4):
                    i = t * 4 + ii
                    xnb = p1n.tile([DH, D], bf16, tag="xnb")
                    nc.scalar.activation(xnb[:], xts[ii][:], AF.Copy,
                                         scale=rc4[:, ii:ii + 1])
                    nc.sync.dma_start(out=xn_dram[t][ts(ii, DH), :], in_=xnb[:])
                # -- DMA-transpose this chunk: [512, 128] -> [128, 512] x8 --
                xnT = xnt_pool.tile([DH, 8, 512], bf16, tag="xnT")
                for ci in range(8):
                    nc.sync.dma_start_transpose(
                        out=xnT[:, ci, :],
                        in_=xn_dram[t][:, ts(ci, DH)])
                # -- qkv matmuls --
                pq = {}
                for blk in range(3):  # 0=q 1=k 2=v
                    ps = ps_qkv.tile([DH, 512], f32, tag="qkv")
                    pq[blk] = ps
                    for ci in range(8):
                        nc.tensor.matmul(
                            ps[:], wTg[:, ci, ts(blk, DH)],
                            xnT[:, ci, :],
                            start=(ci == 0), stop=(ci == 7))
                # q: scaled copy; k: raw copy; v: copy for transposing
                nc.scalar.activation(qsT[t][:], pq[0][:], AF.Copy, scale=SCALE)
                nc.scalar.activation(kT[t][:], pq[1][:], AF.Copy)
                vT = vtp.tile([DH, 512], f32, tag="vT")
                nc.scalar.activation(vT[:], pq[2][:], AF.Copy)
                # feature maps: f(x) = relu(x) + exp(min(x, 0))
                for blk, dst in ((0, qfT), (1, kfT)):
                    src = pq[blk]
                    mn = p3t.tile([DH, 512], f32, tag="mn")
                    nc.vector.tensor_scalar(mn[:], src[:], 0.0, None, OP.min)
                    ex = p3t.tile([DH, 512], f32, tag="ex")
                    nc.scalar.activation(ex[:], mn[:], AF.Exp)
                    rl = p3t.tile([DH, 512], f32, tag="rl")
                    nc.scalar.activation(rl[:], src[:], AF.Relu)
                    acc = nacc[:, t:t + 1] if blk == 1 else None
                    nc.vector.scalar_tensor_tensor(
                        out=dst[:, t0:t0 + 512], in0=ex[:], scalar=1.0,
                        in1=rl[:], op0=OP.mult, op1=OP.add, accum_out=acc)
                # rope (in-place): rot = Pc @ src ; src = src*cos + rot*sin
                for srcT in (qsT[t], kT[t]):
                    rp = ps_rot.tile([DH, 512], f32, tag="rot")
                    nc.tensor.matmul(rp[:], pc_sb[:], srcT[:], start=True,
                                     stop=True)
                    t1 = p3t.tile([DH, 512], f32, tag="t1")
                    nc.gpsimd.tensor_mul(t1[:], srcT[:].bitcast(f32),
                                         cosT[:, n0:n0 + 512])
                    m2 = p3t.tile([DH, 512], f32, tag="m2")
                    nc.vector.tensor_mul(m2[:], rp[:], sinT[:, n0:n0 + 512])
                    nc.gpsimd.tensor_add(srcT[:], t1[:], m2[:])
                # v and kf transposes for this chunk (4 blocks each)
                for bi in range(4):
                    tb = t * 4 + bi
                    tp = ps_tr.tile([DH, DH], f32, tag="tr")
                    nc.tensor.transpose(tp[:], vT[:, ts(bi, DH)], id_sb[:])
                    nc.vector.tensor_copy(v_nat[:, tb, :], tp[:])
                    nc.scalar.activation(v_aug[:, tb, 0:DH], tp[:], AF.Copy)
                    tp2 = ps_tr.tile([DH, DH], f32, tag="tr")
                    nc.tensor.transpose(tp2[:], kfT[:, ts(tb, DH)], id_sb[:])
                    nc.vector.tensor_copy(kf_nat[:, tb, :], tp2[:])

        pclose("xnt")
        pclose("wtg", "rope")
        if stage < 4:
            pclose("nat", "big", "consts", "dram")
            return

        # ---- Phase 4: causal attention + gated retrieval, per batch ----
        ogt_pool = popen("ogt", bufs=1, side="right")
        ogT = ogt_pool.tile([DH, NT], bf16)
        NB = N // DH  # 16 key/query blocks per batch

        with tc.tile_pool(name="expp", bufs=1) as expp, \
             tc.tile_pool(name="ps_sim", bufs=2, space="PSUM") as ps_sim, \
             tc.tile_pool(name="ps_pv", bufs=2, space="PSUM") as ps_pv, \
             tc.tile_pool(name="ps_rt", bufs=1, space="PSUM") as ps_rt, \
             tc.tile_pool(name="ps_tr2", bufs=1, space="PSUM") as ps_tr2, \
             tc.tile_pool(name="p4t", bufs=3) as p4t:
            for b in range(B):
                Bo = b * N
                expt = {}
                for jb in range(NB):
                    al = (jb // 4) * 512
                    d0 = jb * DH
                    W = N - d0
                    et = expp.tile([DH, W], bf16, tag=f"exp{jb}", name=f"exp{jb}")
                    expt[jb] = (et, d0)
                    kblk = kT[(Bo + jb * DH) // 512][:, ts(jb % 4, DH)]
                    for c0 in range(al, N, 1024):
                        cw = min(1024, N - c0)
                        ps = ps_sim.tile([DH, cw], f32, tag="sim")
                        for s0 in range(c0, c0 + cw, 512):
                            qch = qsT[(Bo + s0) // 512]
                            nc.tensor.matmul(
                                ps[:, s0 - c0:s0 - c0 + 512],
                                kblk, qch[:],
                                start=True, stop=True)
                        e0 = max(c0, d0)
                        nc.scalar.activation(et[:, e0 - d0:c0 + cw - d0],
                                             ps[:, e0 - c0:cw], AF.Exp)
                    nc.gpsimd.tensor_mul(et[:, 0:DH], et[:, 0:DH], masku[:])
                for ib in range(NB):
                    pv = ps_pv.tile([DH, 132], f32, tag="pv")
                    for jb in range(ib + 1):
                        et, d0 = expt[jb]
                        nc.tensor.matmul(
                            pv[:, 0:129], et[:, ib * DH - d0:ib * DH - d0 + DH],
                            v_aug[:, b * NB + jb, 0:129],
                            start=(jb == 0), stop=(jb == ib))
                    rt = ps_rt.tile([DH, 132], f32, tag="rt")
                    nc.tensor.matmul(rt[:, 0:129],
                                     qfT[:, Bo + ib * DH:Bo + (ib + 1) * DH],
                                     mem_bf[:, b, 0:129], start=True, stop=True)
                    d1 = p4t.tile([DH, 1], f32, tag="d1")
                    nc.vector.reciprocal(d1[:], pv[:, 128:129])
                    s1 = p4t.tile([DH, 1], f32, tag="s1")
                    nc.vector.tensor_mul(s1[:], d1[:], g128[:])
                    dm = p4t.tile([DH, 1], f32, tag="dm")
                    nc.vector.tensor_scalar(dm[:], rt[:, 128:129], 1e-10, None,
                                            OP.max)
                    d2 = p4t.tile([DH, 1], f32, tag="d2")
                    nc.vector.reciprocal(d2[:], dm[:])
                    s2 = p4t.tile([DH, 1], f32, tag="s2")
                    nc.vector.tensor_mul(s2[:], d2[:], om128[:])
                    t1g = p4t.tile([DH, DH], f32, tag="t1g")
                    nc.vector.tensor_scalar(t1g[:], pv[:, 0:DH], s1[:], None,
                                            OP.mult)
                    og = p4t.tile([DH, DH], f32, tag="og")
                    nc.vector.scalar_tensor_tensor(
                        out=og[:], in0=rt[:, 0:DH], scalar=s2[:], in1=t1g[:],
                        op0=OP.mult, op1=OP.add)
                    tp = ps_tr2.tile([DH, DH], f32, tag="ogtr")
                    nc.tensor.transpose(tp[:], og[:], id_sb[:])
                    nc.vector.tensor_copy(
                        ogT[:, Bo + ib * DH:Bo + (ib + 1) * DH], tp[:])
                if stage >= 6:
                    HCH = N // N_CORES
                    for j in range(N_CORES):
                        nc.sync.dma_start(
                            out=a2a_in[b][ts(j, DH), :],
                            in_=ogT[:, Bo + j * HCH:Bo + (j + 1) * HCH])
                    if stage >= 7:
                        nc.gpsimd.collective_compute(
                            "AllToAll", OP.bypass,
                            replica_groups=[list(range(N_CORES))],
                            ins=[a2a_in[b].opt()], outs=[a2a_out[b].opt()])
                    else:
                        with tc.tile_pool(name=f"bounce{b}", bufs=2) as bounce:
                            for j in range(N_CORES):
                                bt = bounce.tile([DH, HCH], bf16, tag="bt")
                                nc.sync.dma_start(out=bt[:],
                                                  in_=a2a_in[b][ts(j, DH), :])
                                nc.sync.dma_start(out=a2a_out[b][ts(j, DH), :],
                                                  in_=bt[:])

        if stage < 5:
            pclose("nat", "big", "ogt", "consts", "dram")
            return
        # ---- Phase 5: delta-rule memory update, per batch ----
        with tc.tile_pool(name="ps_dr", bufs=2, space="PSUM") as ps_dr, \
             tc.tile_pool(name="ps_nk", bufs=1, space="PSUM") as ps_nk, \
             tc.tile_pool(name="p5t", bufs=3) as p5t:
            for b in range(B):
                Bo = b * N
                nk = ps_nk.tile([DH, DH], f32, tag="nk")
                for tb in range(NB):
                    g = b * NB + tb
                    dr = ps_dr.tile([DH, 132], f32, tag="dr")
                    nc.tensor.matmul(dr[:, 0:129],
                                     kfT[:, Bo + tb * DH:Bo + (tb + 1) * DH],
                                     mem_sb[:, b, 0:129], start=True, stop=True)
                    dm = p5t.tile([DH, 1], f32, tag="dm5")
                    nc.vector.tensor_scalar(dm[:], dr[:, 128:129], 1e-10, None,
                                            OP.max)
                    rd = p5t.tile([DH, 1], f32, tag="rd5")
                    nc.vector.reciprocal(rd[:], dm[:])
                    rdn = p5t.tile([DH, 1], f32, tag="rdn5")
                    nc.vector.tensor_scalar(rdn[:], rd[:], -1.0, None, OP.mult)
                    vn = p5t.tile([DH, DH], f32, tag="vn")
                    nc.vector.scalar_tensor_tensor(
                        out=vn[:], in0=dr[:, 0:DH], scalar=rdn[:],
                        in1=v_nat[:, g, :], op0=OP.mult, op1=OP.add)
                    nc.tensor.matmul(nk[:], kf_nat[:, g, :], vn[:],
                                     start=(tb == 0), stop=(tb == NB - 1),
                                     skip_group_check=True)
                nkv = p5t.tile([DH, DH], f32, tag="nkv")
                nc.vector.tensor_add(nkv[:], nk[:], mem_sb[:, b, 0:DH])
                nc.sync.dma_start(out=nkv_d[b], in_=nkv[:])
                nsum = p5t.tile([DH, 1], f32, tag="nsum")
                nc.vector.tensor_reduce(nsum[:], nacc[:, b * 4:(b + 1) * 4],
                                        AX.X, OP.add)
                nn2 = p5t.tile([DH, 1], f32, tag="nn2")
                nc.vector.tensor_add(nn2[:], nsum[:], mem_sb[:, b, 128:129])
                nc.sync.dma_start(out=nnm_d[b], in_=nn2[:])

        pclose("nat", "big")

        if stage < 6:
            pclose("ogt", "consts", "dram")
            return
        # ---- Phase 6: token-sharded output projection ----
        HCH = N // N_CORES  # 256 tokens per core per batch
        fin = popen("fin", bufs=1)
        wo = fin.tile([DH, 8, D], bf16)
        with tc.tile_pool(name="wost", bufs=2) as wost:
            for ci in range(8):
                wst = wost.tile([DH, D], f32, tag="wst")
                nc.sync.dma_start(out=wst[:], in_=woT_d[ts(ci, DH), :])
                nc.vector.tensor_copy(wo[:, ci, :], wst[:])
        mg = fin.tile([DH, B, 8, HCH], bf16)
        for b in range(B):
            for c in range(8):
                nc.sync.dma_start(out=mg[:, b, c, :],
                                  in_=a2a_out[b][ts(c, DH), :])
        with tc.tile_pool(name="ps_fin", bufs=2, space="PSUM") as ps_fin, \
             tc.tile_pool(name="p6t", bufs=2) as p6t:
            for b in range(B):
                for tb in range(HCH // DH):
                    fp = ps_fin.tile([DH, D], f32, tag="fin")
                    for ci in range(8):
                        for o0 in range(0, D, 512):
                            nc.tensor.matmul(
                                fp[:, o0:o0 + 512],
                                mg[:, b, ci, ts(tb, DH)],
                                wo[:, ci, o0:o0 + 512],
                                start=(ci == 0), stop=(ci == 7),
                                skip_group_check=True)
                    ot = p6t.tile([DH, D], f32, tag="ot")
                    nc.vector.tensor_copy(ot[:], fp[:])
                    nc.sync.dma_start(out=out_d[ts(b * 2 + tb, DH), :], in_=ot[:])

        pclose("fin", "ogt", "consts", "dram")

    with tile.TileContext(nc) as tc:
        _emit(tc)
    nc.compile()
    return nc


def _get_nc():
    if "nc" not in _cache:
        import os
        stage = int(os.environ.get("BASS_KERNEL_STAGE", "99"))
        try:
            _cache["nc"] = _build_nc(stage)
        except _StageDone as e:
            raise RuntimeError("unreachable")
    return _cache["nc"]


def kernel(x, gamma, w_qkv, w_out, head_gates, mem_kv, mem_norm):
    from concourse.bass_utils import run_bass_kernel_spmd

    x = np.asarray(x, dtype=np.float32)
    gamma = np.asarray(gamma, dtype=np.float32)
    w_qkv = np.asarray(w_qkv, dtype=np.float32)
    w_out = np.asarray(w_out, dtype=np.float32)
    head_gates = np.asarray(head_gates, dtype=np.float32)
    mem_kv = np.asarray(mem_kv, dtype=np.float32)
    mem_norm = np.asarray(mem_norm, dtype=np.float32)

    nc = _get_nc()
    cosT, sinT, pc, masku, ident = _host_constants()

    x_flat = np.ascontiguousarray(x.reshape(NT, D))
    gamma2 = np.ascontiguousarray(gamma.reshape(D, 1))
    woT = np.ascontiguousarray(w_out.T)

    in_maps = []
    for h in range(N_CORES):
        wq = w_qkv[h * DH:(h + 1) * DH]
        wk = w_qkv[D + h * DH:D + (h + 1) * DH]
        wv = w_qkv[2 * D + h * DH:2 * D + (h + 1) * DH]
        wT = np.ascontiguousarray(np.concatenate([wq, wk, wv], 0).T)  # [D, 384]
        mem_aug = np.zeros((B, DH, 132), dtype=np.float32)
        mem_aug[:, :, 0:DH] = mem_kv[:, h]
        mem_aug[:, :, DH] = mem_norm[:, h]
        in_maps.append({
            "x": x_flat, "wT": wT, "gamma": gamma2, "w_outT": woT,
            "cosT": cosT, "sinT": sinT, "pc": pc, "masku": masku,
            "ident": ident, "mem_aug": mem_aug,
            "hg": head_gates[h].reshape(1, 1),
        })

    res = run_bass_kernel_spmd(nc, in_maps, core_ids=list(range(N_CORES)),
                               **_cache.get("run_kwargs", {}))
    _cache["last_result"] = res

    out = np.empty((NT, D), dtype=np.float32)
    new_kv = np.empty((B, H, DH, DH), dtype=np.float32)
    new_norm = np.empty((B, H, DH), dtype=np.float32)
    HCH = N // N_CORES
    for h in range(N_CORES):
        r = res.results[h]
        out[h * HCH:(h + 1) * HCH] = r["out"][0:HCH]
        out[N + h * HCH:N + (h + 1) * HCH] = r["out"][HCH:2 * HCH]
        new_kv[:, h] = r["new_kv"]
        new_norm[:, h] = r["new_norm"].reshape(B, DH)
    return out.reshape(B, N, D), new_kv, new_norm
